# revision 50
# baseline (speedup 1.0000x reference)
"""Conv2d-via-Linear Trainium2 kernel.

The problem's [16,30,30,3,64,64] weight is (for the reference's
setup_inputs) a structured-sparse replication of a single 5x5/stride-2
conv kernel w0 [16,3,5,5]:  big[:, oh, ow, :, 2oh:2oh+5, 2ow:2ow+5] = w0.
So out = x2 @ w2.T + bias is exactly Conv2d(x, w0, stride=2) + b0.

Device strategy (8 NeuronCores, batch-parallel, 8 images per core):
  - Host lays out each core's batch shard as a 75-row im2col operand
    X[(c,kh,kw), (b, oh, ow)] with the (kh>>1, kw>>1) spatial shifts baked
    in as flat offsets; a 76th all-ones row folds the bias into the
    matmul (weight row 75 holds b0).
  - Single-pass bf16 (PSUM accumulates fp32; rel err ~3e-3, well under
    the 2e-2 gate) instead of a 3-pass hi/lo split: halves input bytes
    and cuts matmul count 3x.
  - The DGE splits a DMA's descriptors (one per partition row) into
    equal contiguous blocks across k engines where k = the largest
    divisor of the row count that is <= 16. 76 rows -> 4 engines (76 =
    4*19, ~90 GB/s); 80 rows -> 16 engines (360 GB/s). So all loads pad
    the partition dim to 80 (rows 76..79 zero weights/data).
  - Input split into 4 chunks (2 images each, [80, 4KB rows])
    alternating the sync/scalar HWDGE rings so matmuls pipeline behind
    the stream (~230 GB/s effective; the stream rate is insensitive to
    descriptor size, so fine chunks win); the [80,32] weight block
    rides as trailing columns of chunk 0.
  - One fp32 matmul (K=76, M=32, cols 16..31 zero) per (image, oh-half)
    chunk; PE column-tiling packs 4 chunks into the 4 col-groups of one
    PSUM bank.
  - Evacuation casts PSUM fp32 -> SBUF bf16 (bias already folded in),
    alternating DVE/ACT engines; each [128,450] group DMAs out as soon
    as its evac lands; the host keeps rows :16 of each 32-group.

If the weight/bias do not have the replicated-conv structure (never the
case for the real reference inputs), falls back to the dense matmul on
host so the result is still correct.
"""

import numpy as np

B, C, H, W = 64, 3, 64, 64
O, KK, S = 16, 5, 2
OH = OW = 30
NCORES = 8
BPC = B // NCORES  # images per core

HB = H // 2  # 32  (h2 dim)
WB = W // 2  # 32  (w2 dim)
XQ_LEN = 2 * C * 2 * BPC * HB * WB  # 98304
XQ_PAD = XQ_LEN + 512  # shifted reads spill <= 2*256+1 past plane ends

# (khp, kwp) -> partition-block layout. kh = 2*khh + khp, kw = 2*kwh + kwp.
_BLOCKS = []
_p0 = 0
for _khp in (0, 1):
    for _kwp in (0, 1):
        _nkh = 3 - _khp
        _nkw = 3 - _kwp
        _BLOCKS.append((_khp, _kwp, _p0, _nkh, _nkw))
        _p0 += _nkh * _nkw * C
NPART = _p0  # 75
NPARTB = 80  # 75 taps + ones row (folded bias) + 4 zero pad rows; 80 = 16*5
             # so the DGE spreads every load across all 16 DMA engines

# element strides inside flat xq [hp, c, wp, b, h2, w2]
_ST_WP = BPC * HB * WB        # 8192
_ST_C = 2 * _ST_WP            # 16384
_ST_HP = C * _ST_C            # 49152

# per-partition source offsets into flat xq, and (c, kh, kw) per partition
_SRC_OFF = np.empty(NPART, dtype=np.int64)
_PART_CKHKW = []
for _khp, _kwp, _pp0, _nkh, _nkw in _BLOCKS:
    _pi = _pp0
    for _khh in range(_nkh):
        for _kwh in range(_nkw):
            for _c in range(C):
                _SRC_OFF[_pi] = (_khp * _ST_HP + _c * _ST_C + _kwp * _ST_WP
                                 + _khh * HB + _kwh)
                _PART_CKHKW.append((_c, 2 * _khh + _khp, 2 * _kwh + _kwp))
                _pi += 1

IMG = HB * WB  # 1024 elems per image per partition
CHUNKS = [(0, 2), (2, 4), (4, 6), (6, 8)]  # image ranges, one group each
NCHUNK = len(CHUNKS)

_NC_CACHE = {}
LAST_RESULT = None


def _install_trace_shim():
    """Make bass_utils' trace path importable even when antenv.axon_hooks
    is absent (it is in this container). Harmless if tracing is off."""
    import sys, types
    try:
        import antenv.axon_hooks  # noqa: F401
        return
    except ImportError:
        pass
    mod = types.ModuleType("antenv.axon_hooks")
    hook = [None]
    mod.set_axon_ntff_profile_hook = lambda h: hook.__setitem__(0, h)
    mod.get_axon_ntff_profile_hook = lambda: hook[0]
    sys.modules["antenv.axon_hooks"] = mod
    try:
        from trn_agent_boot.trn_boot import _ntff_profile_via_ctypes
        hook[0] = _ntff_profile_via_ctypes("/opt/axon/libaxon_pjrt.so")
    except Exception:
        pass


def _structure_ok(weight, w0, bias, b0):
    """Exact check that `weight` is w0 replicated per output position and
    everything else zero, and that bias is b0 repeated per position."""
    try:
        from numpy.lib.stride_tricks import as_strided
        s = weight.strides
        blocks = as_strided(
            weight,
            shape=(OH, OW, O, C, KK, KK),
            strides=(s[1] + S * s[4], s[2] + S * s[5], s[0], s[3], s[4], s[5]),
        )
        if not (blocks == w0[None, None]).all():
            return False
        if np.count_nonzero(weight) != OH * OW * np.count_nonzero(w0):
            return False
        if not (bias[0].reshape(O, OH * OW) == b0[:, None]).all():
            return False
        return True
    except Exception:
        return False


def _build_nc():
    import concourse.mybir as mybir
    import concourse.tile as tile
    from concourse import bacc

    f32 = mybir.dt.float32
    bf16 = mybir.dt.bfloat16
    nc = bacc.Bacc(None, target_bir_lowering=False)
    with tile.TileContext(nc) as tc:
        with tc.tile_pool(name="dram", bufs=1, space="DRAM") as dram:
            # chunk 0 carries the [80,32] weight block as trailing columns
            xt_d = [
                dram.tile([NPARTB, IMG * (e - s) + (32 if c == 0 else 0)],
                          bf16, kind="ExternalInput", name=f"xt{c}",
                          uniquify=False)
                for c, (s, e) in enumerate(CHUNKS)
            ]
            out = dram.tile([4, 128, 15 * OW], bf16, kind="ExternalOutput",
                            name="out", uniquify=False)

            with (
                tc.tile_pool(name="xdata", bufs=1) as xpool,
                tc.tile_pool(name="evac", bufs=4) as evacp,
                tc.tile_pool(name="psum", bufs=4, space="PSUM") as psump,
            ):
                # everything on the sync ring: strict global FIFO means the
                # output transfers queue behind ALL input chunks, so they
                # can never interleave with (and drag out) the input stream
                xts = []
                for c in range(NCHUNK):
                    t = xpool.tile(list(xt_d[c].shape), bf16, name=f"xs{c}")
                    nc.sync.dma_start(t[:], xt_d[c][:])
                    xts.append(t)
                wsb = xts[0][:, 2 * IMG:2 * IMG + 32]  # [80,32], 16..31 zero

                for g in range(4):
                    xv = xts[g][:, 0:2 * IMG].rearrange(
                        "p (b h w) -> p b h w", b=2, h=HB, w=WB)
                    ps = psump.tile([128, 15, OW], f32, tag="ps")
                    for j in range(4):
                        bl, hs = j >> 1, j & 1
                        rhs = xv[:, bl, 15 * hs:15 * hs + 15, 0:OW]
                        nc.tensor.matmul(
                            ps[32 * j:32 * j + 32],
                            wsb,
                            rhs,
                            start=True,
                            stop=True,
                            skip_group_check=True,
                            tile_position=(0, 32 * j),
                        )
                    # evacuate with fp32->bf16 cast (bias already folded in)
                    ev = evacp.tile([128, 15 * OW], bf16, tag="ev")
                    src = ps[:].rearrange("p a b -> p (a b)")
                    if g < 3:
                        nc.vector.tensor_scalar_add(ev[:], src, 0.0)
                    else:
                        # last group: halve the critical-tail evac by
                        # splitting across DVE and ACT (ACT's one-time
                        # table load runs at program start, off-path)
                        nc.vector.tensor_scalar_add(
                            ev[:, 0:225], src[:, 0:225], 0.0)
                        nc.scalar.copy(ev[:, 225:450], src[:, 225:450])
                    nc.sync.dma_start(out[g, :, :], ev[:])
    nc.compile()
    return nc


def kernel(x, weight, bias):
    global LAST_RESULT
    x = np.ascontiguousarray(np.asarray(x), dtype=np.float32)
    weight = np.asarray(weight)
    bias = np.ascontiguousarray(np.asarray(bias), dtype=np.float32)

    w0 = np.ascontiguousarray(weight[:, 0, 0, :, :KK, :KK], dtype=np.float32)
    b0 = bias[0].reshape(O, OH * OW)[:, 0].copy()

    if not _structure_ok(weight, w0, bias, b0):
        # Unstructured weight: fall back to the dense matmul on host.
        x2 = x.reshape(B, -1)
        w2 = np.asarray(weight, dtype=np.float32).reshape(O * OH * OW, -1)
        return (x2 @ w2.T + bias).reshape(B, O, OH, OW).astype(np.float32)

    _install_trace_shim()

    # host layout prep --------------------------------------------------
    # xq[core][hp, c, wp, b, h2, w2] = x[core*8 + b, c, 2*h2+hp, 2*w2+wp]
    xs = x.reshape(NCORES, BPC, C, HB, 2, WB, 2)
    xq = np.ascontiguousarray(xs.transpose(0, 4, 2, 6, 1, 3, 5)).reshape(NCORES, XQ_LEN)
    xqp = np.zeros((NCORES, XQ_PAD), dtype=np.float32)
    xqp[:, :XQ_LEN] = xq
    # X[(p), b*1024 + t] = xq[src_off[p] + b*1024 + t]
    idx = (_SRC_OFF[:, None, None]
           + np.arange(BPC)[None, :, None] * IMG
           + np.arange(IMG)[None, None, :])
    import ml_dtypes
    bf = ml_dtypes.bfloat16
    xbig = np.zeros((NCORES, NPARTB, BPC * IMG), dtype=bf)
    xbig[:, :NPART] = xqp[:, idx.reshape(NPART, -1)]
    xbig[:, NPART] = np.float32(1.0)  # folded-bias ones row; 76..79 stay 0

    # w0r[p, o] with p ordered (khp, kwp, khh, kwh, c); row 75 = b0 (bias
    # rides the ones row); cols 16..31 zero
    w0f = np.zeros((NPARTB, 32), dtype=np.float32)
    for p, (c, kh, kw) in enumerate(_PART_CKHKW):
        w0f[p, :O] = w0[:, c, kh, kw]
    w0f[NPART, :O] = b0
    w0r = w0f.astype(bf)

    # device run --------------------------------------------------------
    if "nc" not in _NC_CACHE:
        _NC_CACHE["nc"] = _build_nc()
    nc = _NC_CACHE["nc"]

    from concourse.bass_utils import run_bass_kernel_spmd

    in_maps = []
    for i in range(NCORES):
        m = {}
        for c, (s, e) in enumerate(CHUNKS):
            sl = xbig[i, :, IMG * s:IMG * e]
            if c == 0:
                sl = np.concatenate([sl, w0r], axis=1)
            m[f"xt{c}"] = np.ascontiguousarray(sl)
        in_maps.append(m)
    res = run_bass_kernel_spmd(nc, in_maps, core_ids=list(range(NCORES)))
    LAST_RESULT = res

    out = np.empty((B, O, OH, OW), dtype=np.float32)
    for i in range(NCORES):
        # device out: [g, 32*j + o, oh'*30 + ow] with
        # b = 2*g + (j>>1), oh = 15*(j&1) + oh'
        dv = res.results[i]["out"].astype(np.float32).reshape(4, 4, 32, 15, OW)
        for g in range(4):
            for j in range(4):
                b, hs = 2 * g + (j >> 1), j & 1
                out[i * BPC + b, :, 15 * hs:15 * hs + 15, :] = dv[g, j, :O]
    return out


# revision 52
# speedup vs baseline: 1.0023x; 1.0023x over previous
"""Conv2d-via-Linear Trainium2 kernel.

The problem's [16,30,30,3,64,64] weight is (for the reference's
setup_inputs) a structured-sparse replication of a single 5x5/stride-2
conv kernel w0 [16,3,5,5]:  big[:, oh, ow, :, 2oh:2oh+5, 2ow:2ow+5] = w0.
So out = x2 @ w2.T + bias is exactly Conv2d(x, w0, stride=2) + b0.

Device strategy (8 NeuronCores, batch-parallel, 8 images per core):
  - Host lays out each core's batch shard as a 75-row im2col operand
    X[(c,kh,kw), (b, oh, ow)] with the (kh>>1, kw>>1) spatial shifts baked
    in as flat offsets; a 76th all-ones row folds the bias into the
    matmul (weight row 75 holds b0).
  - Single-pass bf16 (PSUM accumulates fp32; rel err ~3e-3, well under
    the 2e-2 gate) instead of a 3-pass hi/lo split: halves input bytes
    and cuts matmul count 3x.
  - The DGE splits a DMA's descriptors (one per partition row) into
    equal contiguous blocks across k engines where k = the largest
    divisor of the row count that is <= 16. 76 rows -> 4 engines (76 =
    4*19, ~90 GB/s); 80 rows -> 16 engines (360 GB/s). So all loads pad
    the partition dim to 80 (rows 76..79 zero weights/data).
  - Input split into 4 chunks (2 images each, [80, 4KB rows])
    alternating the sync/scalar HWDGE rings so matmuls pipeline behind
    the stream (~230 GB/s effective; the stream rate is insensitive to
    descriptor size, so fine chunks win); the [80,32] weight block
    rides as trailing columns of chunk 0.
  - One fp32 matmul (K=76, M=32, cols 16..31 zero) per (image, oh-half)
    chunk; PE column-tiling packs 4 chunks into the 4 col-groups of one
    PSUM bank.
  - Evacuation casts PSUM fp32 -> SBUF bf16 (bias already folded in),
    alternating DVE/ACT engines; each [128,450] group DMAs out as soon
    as its evac lands; the host keeps rows :16 of each 32-group.

If the weight/bias do not have the replicated-conv structure (never the
case for the real reference inputs), falls back to the dense matmul on
host so the result is still correct.
"""

import numpy as np

B, C, H, W = 64, 3, 64, 64
O, KK, S = 16, 5, 2
OH = OW = 30
NCORES = 8
BPC = B // NCORES  # images per core

HB = H // 2  # 32  (h2 dim)
WB = W // 2  # 32  (w2 dim)
XQ_LEN = 2 * C * 2 * BPC * HB * WB  # 98304
XQ_PAD = XQ_LEN + 512  # shifted reads spill <= 2*256+1 past plane ends

# (khp, kwp) -> partition-block layout. kh = 2*khh + khp, kw = 2*kwh + kwp.
_BLOCKS = []
_p0 = 0
for _khp in (0, 1):
    for _kwp in (0, 1):
        _nkh = 3 - _khp
        _nkw = 3 - _kwp
        _BLOCKS.append((_khp, _kwp, _p0, _nkh, _nkw))
        _p0 += _nkh * _nkw * C
NPART = _p0  # 75
NPARTB = NPART + 1  # 75 taps + ones row (folded bias) = 76. Each load is
# split into [0:64] + [64:76] partition ranges: the DGE spreads a DMA
# across k engines where k = largest divisor of its descriptor count
# <= 16, so 64 -> 16 engines and 12 -> 12 engines (76 alone would use 4)

# element strides inside flat xq [hp, c, wp, b, h2, w2]
_ST_WP = BPC * HB * WB        # 8192
_ST_C = 2 * _ST_WP            # 16384
_ST_HP = C * _ST_C            # 49152

# per-partition source offsets into flat xq, and (c, kh, kw) per partition
_SRC_OFF = np.empty(NPART, dtype=np.int64)
_PART_CKHKW = []
for _khp, _kwp, _pp0, _nkh, _nkw in _BLOCKS:
    _pi = _pp0
    for _khh in range(_nkh):
        for _kwh in range(_nkw):
            for _c in range(C):
                _SRC_OFF[_pi] = (_khp * _ST_HP + _c * _ST_C + _kwp * _ST_WP
                                 + _khh * HB + _kwh)
                _PART_CKHKW.append((_c, 2 * _khh + _khp, 2 * _kwh + _kwp))
                _pi += 1

IMG = HB * WB  # 1024 elems per image per partition
CHUNKS = [(0, 2), (2, 4), (4, 6), (6, 8)]  # image ranges, one group each
NCHUNK = len(CHUNKS)

_NC_CACHE = {}
LAST_RESULT = None


def _install_trace_shim():
    """Make bass_utils' trace path importable even when antenv.axon_hooks
    is absent (it is in this container). Harmless if tracing is off."""
    import sys, types
    try:
        import antenv.axon_hooks  # noqa: F401
        return
    except ImportError:
        pass
    mod = types.ModuleType("antenv.axon_hooks")
    hook = [None]
    mod.set_axon_ntff_profile_hook = lambda h: hook.__setitem__(0, h)
    mod.get_axon_ntff_profile_hook = lambda: hook[0]
    sys.modules["antenv.axon_hooks"] = mod
    try:
        from trn_agent_boot.trn_boot import _ntff_profile_via_ctypes
        hook[0] = _ntff_profile_via_ctypes("/opt/axon/libaxon_pjrt.so")
    except Exception:
        pass


def _structure_ok(weight, w0, bias, b0):
    """Exact check that `weight` is w0 replicated per output position and
    everything else zero, and that bias is b0 repeated per position."""
    try:
        from numpy.lib.stride_tricks import as_strided
        s = weight.strides
        blocks = as_strided(
            weight,
            shape=(OH, OW, O, C, KK, KK),
            strides=(s[1] + S * s[4], s[2] + S * s[5], s[0], s[3], s[4], s[5]),
        )
        if not (blocks == w0[None, None]).all():
            return False
        if np.count_nonzero(weight) != OH * OW * np.count_nonzero(w0):
            return False
        if not (bias[0].reshape(O, OH * OW) == b0[:, None]).all():
            return False
        return True
    except Exception:
        return False


def _build_nc():
    import concourse.mybir as mybir
    import concourse.tile as tile
    from concourse import bacc

    f32 = mybir.dt.float32
    bf16 = mybir.dt.bfloat16
    nc = bacc.Bacc(None, target_bir_lowering=False)
    with tile.TileContext(nc) as tc:
        with tc.tile_pool(name="dram", bufs=1, space="DRAM") as dram:
            # chunk 0 carries the [80,32] weight block as trailing columns
            xt_d = [
                dram.tile([NPARTB, IMG * (e - s) + (32 if c == 0 else 0)],
                          bf16, kind="ExternalInput", name=f"xt{c}",
                          uniquify=False)
                for c, (s, e) in enumerate(CHUNKS)
            ]
            out = dram.tile([4, 128, 15 * OW], bf16, kind="ExternalOutput",
                            name="out", uniquify=False)

            with (
                tc.tile_pool(name="xdata", bufs=1) as xpool,
                tc.tile_pool(name="evac", bufs=4) as evacp,
                tc.tile_pool(name="psum", bufs=4, space="PSUM") as psump,
            ):
                # everything on the sync ring: strict global FIFO means the
                # output transfers queue behind ALL input chunks, so they
                # can never interleave with (and drag out) the input stream
                xts = []
                for c in range(NCHUNK):
                    t = xpool.tile(list(xt_d[c].shape), bf16, name=f"xs{c}")
                    nc.sync.dma_start(t[0:64], xt_d[c][0:64])
                    nc.sync.dma_start(t[64:NPARTB], xt_d[c][64:NPARTB])
                    xts.append(t)
                wsb = xts[0][:, 2 * IMG:2 * IMG + 32]  # [80,32], 16..31 zero

                for g in range(4):
                    xv = xts[g][:, 0:2 * IMG].rearrange(
                        "p (b h w) -> p b h w", b=2, h=HB, w=WB)
                    ps = psump.tile([128, 15, OW], f32, tag="ps")
                    for j in range(4):
                        bl, hs = j >> 1, j & 1
                        rhs = xv[:, bl, 15 * hs:15 * hs + 15, 0:OW]
                        nc.tensor.matmul(
                            ps[32 * j:32 * j + 32],
                            wsb,
                            rhs,
                            start=True,
                            stop=True,
                            skip_group_check=True,
                            tile_position=(0, 32 * j),
                        )
                    # evacuate with fp32->bf16 cast (bias already folded in);
                    # all on DVE: ACT copy would pull in a 1.3us table load
                    ev = evacp.tile([128, 15 * OW], bf16, tag="ev")
                    src = ps[:].rearrange("p a b -> p (a b)")
                    nc.vector.tensor_scalar_add(ev[:], src, 0.0)
                    nc.sync.dma_start(out[g, :, :], ev[:])
    nc.compile()
    return nc


def kernel(x, weight, bias):
    global LAST_RESULT
    x = np.ascontiguousarray(np.asarray(x), dtype=np.float32)
    weight = np.asarray(weight)
    bias = np.ascontiguousarray(np.asarray(bias), dtype=np.float32)

    w0 = np.ascontiguousarray(weight[:, 0, 0, :, :KK, :KK], dtype=np.float32)
    b0 = bias[0].reshape(O, OH * OW)[:, 0].copy()

    if not _structure_ok(weight, w0, bias, b0):
        # Unstructured weight: fall back to the dense matmul on host.
        x2 = x.reshape(B, -1)
        w2 = np.asarray(weight, dtype=np.float32).reshape(O * OH * OW, -1)
        return (x2 @ w2.T + bias).reshape(B, O, OH, OW).astype(np.float32)

    _install_trace_shim()

    # host layout prep --------------------------------------------------
    # xq[core][hp, c, wp, b, h2, w2] = x[core*8 + b, c, 2*h2+hp, 2*w2+wp]
    xs = x.reshape(NCORES, BPC, C, HB, 2, WB, 2)
    xq = np.ascontiguousarray(xs.transpose(0, 4, 2, 6, 1, 3, 5)).reshape(NCORES, XQ_LEN)
    xqp = np.zeros((NCORES, XQ_PAD), dtype=np.float32)
    xqp[:, :XQ_LEN] = xq
    # X[(p), b*1024 + t] = xq[src_off[p] + b*1024 + t]
    idx = (_SRC_OFF[:, None, None]
           + np.arange(BPC)[None, :, None] * IMG
           + np.arange(IMG)[None, None, :])
    import ml_dtypes
    bf = ml_dtypes.bfloat16
    xbig = np.empty((NCORES, NPARTB, BPC * IMG), dtype=bf)
    xbig[:, :NPART] = xqp[:, idx.reshape(NPART, -1)]
    xbig[:, NPART] = np.float32(1.0)  # folded-bias ones row

    # w0r[p, o] with p ordered (khp, kwp, khh, kwh, c); row 75 = b0 (bias
    # rides the ones row); cols 16..31 zero
    w0f = np.zeros((NPARTB, 32), dtype=np.float32)
    for p, (c, kh, kw) in enumerate(_PART_CKHKW):
        w0f[p, :O] = w0[:, c, kh, kw]
    w0f[NPART, :O] = b0
    w0r = w0f.astype(bf)

    # device run --------------------------------------------------------
    if "nc" not in _NC_CACHE:
        _NC_CACHE["nc"] = _build_nc()
    nc = _NC_CACHE["nc"]

    from concourse.bass_utils import run_bass_kernel_spmd

    in_maps = []
    for i in range(NCORES):
        m = {}
        for c, (s, e) in enumerate(CHUNKS):
            sl = xbig[i, :, IMG * s:IMG * e]
            if c == 0:
                sl = np.concatenate([sl, w0r], axis=1)
            m[f"xt{c}"] = np.ascontiguousarray(sl)
        in_maps.append(m)
    res = run_bass_kernel_spmd(nc, in_maps, core_ids=list(range(NCORES)))
    LAST_RESULT = res

    out = np.empty((B, O, OH, OW), dtype=np.float32)
    for i in range(NCORES):
        # device out: [g, 32*j + o, oh'*30 + ow] with
        # b = 2*g + (j>>1), oh = 15*(j&1) + oh'
        dv = res.results[i]["out"].astype(np.float32).reshape(4, 4, 32, 15, OW)
        for g in range(4):
            for j in range(4):
                b, hs = 2 * g + (j >> 1), j & 1
                out[i * BPC + b, :, 15 * hs:15 * hs + 15, :] = dv[g, j, :O]
    return out


# revision 53
# speedup vs baseline: 1.0819x; 1.0794x over previous
"""Conv2d-via-Linear Trainium2 kernel.

The problem's [16,30,30,3,64,64] weight is (for the reference's
setup_inputs) a structured-sparse replication of a single 5x5/stride-2
conv kernel w0 [16,3,5,5]:  big[:, oh, ow, :, 2oh:2oh+5, 2ow:2ow+5] = w0.
So out = x2 @ w2.T + bias is exactly Conv2d(x, w0, stride=2) + b0.

Device strategy (8 NeuronCores, batch-parallel, 8 images per core):
  - Host lays out each core's batch shard as a 75-row im2col operand
    X[(c,kh,kw), (b, oh, ow)] with the (kh>>1, kw>>1) spatial shifts baked
    in as flat offsets; a 76th all-ones row folds the bias into the
    matmul (weight row 75 holds b0).
  - Single-pass bf16 (PSUM accumulates fp32; rel err ~3e-3, well under
    the 2e-2 gate) instead of a 3-pass hi/lo split: halves input bytes
    and cuts matmul count 3x.
  - The DGE splits a DMA's descriptors (one per partition row) into
    equal contiguous blocks across k engines where k = the largest
    divisor of the row count that is <= 16. 76 rows -> 4 engines (76 =
    4*19, ~90 GB/s); 80 rows -> 16 engines (360 GB/s). So all loads pad
    the partition dim to 80 (rows 76..79 zero weights/data).
  - Input split into 4 chunks (2 images each, [80, 4KB rows])
    alternating the sync/scalar HWDGE rings so matmuls pipeline behind
    the stream (~230 GB/s effective; the stream rate is insensitive to
    descriptor size, so fine chunks win); the [80,32] weight block
    rides as trailing columns of chunk 0.
  - One fp32 matmul (K=76, M=32, cols 16..31 zero) per (image, oh-half)
    chunk; PE column-tiling packs 4 chunks into the 4 col-groups of one
    PSUM bank.
  - Evacuation casts PSUM fp32 -> SBUF bf16 (bias already folded in),
    alternating DVE/ACT engines; each [128,450] group DMAs out as soon
    as its evac lands; the host keeps rows :16 of each 32-group.

If the weight/bias do not have the replicated-conv structure (never the
case for the real reference inputs), falls back to the dense matmul on
host so the result is still correct.
"""

import numpy as np

B, C, H, W = 64, 3, 64, 64
O, KK, S = 16, 5, 2
OH = OW = 30
NCORES = 8
BPC = B // NCORES  # images per core

HB = H // 2  # 32  (h2 dim)
WB = W // 2  # 32  (w2 dim)
XQ_LEN = 2 * C * 2 * BPC * HB * WB  # 98304
XQ_PAD = XQ_LEN + 512  # shifted reads spill <= 2*256+1 past plane ends

# (khp, kwp) -> partition-block layout. kh = 2*khh + khp, kw = 2*kwh + kwp.
_BLOCKS = []
_p0 = 0
for _khp in (0, 1):
    for _kwp in (0, 1):
        _nkh = 3 - _khp
        _nkw = 3 - _kwp
        _BLOCKS.append((_khp, _kwp, _p0, _nkh, _nkw))
        _p0 += _nkh * _nkw * C
NPART = _p0  # 75
NPARTB = 80  # 75 taps + ones row (folded bias) + 4 zero pad rows; 80 = 16*5
             # so the DGE spreads every load across all 16 DMA engines

# element strides inside flat xq [hp, c, wp, b, h2, w2]
_ST_WP = BPC * HB * WB        # 8192
_ST_C = 2 * _ST_WP            # 16384
_ST_HP = C * _ST_C            # 49152

# per-partition source offsets into flat xq, and (c, kh, kw) per partition
_SRC_OFF = np.empty(NPART, dtype=np.int64)
_PART_CKHKW = []
for _khp, _kwp, _pp0, _nkh, _nkw in _BLOCKS:
    _pi = _pp0
    for _khh in range(_nkh):
        for _kwh in range(_nkw):
            for _c in range(C):
                _SRC_OFF[_pi] = (_khp * _ST_HP + _c * _ST_C + _kwp * _ST_WP
                                 + _khh * HB + _kwh)
                _PART_CKHKW.append((_c, 2 * _khh + _khp, 2 * _kwh + _kwp))
                _pi += 1

IMG = HB * WB  # 1024 elems per image per partition
CHUNKS = [(0, 2), (2, 4), (4, 6), (6, 8)]  # image ranges, one group each
NCHUNK = len(CHUNKS)

_NC_CACHE = {}
LAST_RESULT = None


def _install_trace_shim():
    """Make bass_utils' trace path importable even when antenv.axon_hooks
    is absent (it is in this container). Harmless if tracing is off."""
    import sys, types
    try:
        import antenv.axon_hooks  # noqa: F401
        return
    except ImportError:
        pass
    mod = types.ModuleType("antenv.axon_hooks")
    hook = [None]
    mod.set_axon_ntff_profile_hook = lambda h: hook.__setitem__(0, h)
    mod.get_axon_ntff_profile_hook = lambda: hook[0]
    sys.modules["antenv.axon_hooks"] = mod
    try:
        from trn_agent_boot.trn_boot import _ntff_profile_via_ctypes
        hook[0] = _ntff_profile_via_ctypes("/opt/axon/libaxon_pjrt.so")
    except Exception:
        pass


def _structure_ok(weight, w0, bias, b0):
    """Exact check that `weight` is w0 replicated per output position and
    everything else zero, and that bias is b0 repeated per position."""
    try:
        from numpy.lib.stride_tricks import as_strided
        s = weight.strides
        blocks = as_strided(
            weight,
            shape=(OH, OW, O, C, KK, KK),
            strides=(s[1] + S * s[4], s[2] + S * s[5], s[0], s[3], s[4], s[5]),
        )
        if not (blocks == w0[None, None]).all():
            return False
        if np.count_nonzero(weight) != OH * OW * np.count_nonzero(w0):
            return False
        if not (bias[0].reshape(O, OH * OW) == b0[:, None]).all():
            return False
        return True
    except Exception:
        return False


def _build_nc():
    import concourse.mybir as mybir
    import concourse.tile as tile
    from concourse import bacc

    f32 = mybir.dt.float32
    bf16 = mybir.dt.bfloat16
    nc = bacc.Bacc(None, target_bir_lowering=False)
    with tile.TileContext(nc) as tc:
        with tc.tile_pool(name="dram", bufs=1, space="DRAM") as dram:
            # chunk 0 carries the [80,32] weight block as trailing columns
            xt_d = [
                dram.tile([NPARTB, IMG * (e - s) + (32 if c == 0 else 0)],
                          bf16, kind="ExternalInput", name=f"xt{c}",
                          uniquify=False)
                for c, (s, e) in enumerate(CHUNKS)
            ]
            out = dram.tile([4, 128, 15 * OW], bf16, kind="ExternalOutput",
                            name="out", uniquify=False)

            with (
                tc.tile_pool(name="xdata", bufs=1) as xpool,
                tc.tile_pool(name="evac", bufs=4) as evacp,
                tc.tile_pool(name="psum", bufs=4, space="PSUM") as psump,
            ):
                # everything on the sync ring: strict global FIFO means the
                # output transfers queue behind ALL input chunks, so they
                # can never interleave with (and drag out) the input stream
                xts = []
                for c in range(NCHUNK):
                    t = xpool.tile(list(xt_d[c].shape), bf16, name=f"xs{c}")
                    nc.sync.dma_start(t[:], xt_d[c][:])
                    xts.append(t)
                wsb = xts[0][:, 2 * IMG:2 * IMG + 32]  # [80,32], 16..31 zero

                for g in range(4):
                    xv = xts[g][:, 0:2 * IMG].rearrange(
                        "p (b h w) -> p b h w", b=2, h=HB, w=WB)
                    ps = psump.tile([128, 15, OW], f32, tag="ps")
                    for j in range(4):
                        bl, hs = j >> 1, j & 1
                        rhs = xv[:, bl, 15 * hs:15 * hs + 15, 0:OW]
                        nc.tensor.matmul(
                            ps[32 * j:32 * j + 32],
                            wsb,
                            rhs,
                            start=True,
                            stop=True,
                            skip_group_check=True,
                            tile_position=(0, 32 * j),
                        )
                    # evacuate with fp32->bf16 cast (bias already folded in);
                    # all on DVE: ACT copy would pull in a 1.3us table load
                    ev = evacp.tile([128, 15 * OW], bf16, tag="ev")
                    src = ps[:].rearrange("p a b -> p (a b)")
                    nc.vector.tensor_scalar_add(ev[:], src, 0.0)
                    nc.sync.dma_start(out[g, :, :], ev[:])
    nc.compile()
    return nc


def kernel(x, weight, bias):
    global LAST_RESULT
    x = np.ascontiguousarray(np.asarray(x), dtype=np.float32)
    weight = np.asarray(weight)
    bias = np.ascontiguousarray(np.asarray(bias), dtype=np.float32)

    w0 = np.ascontiguousarray(weight[:, 0, 0, :, :KK, :KK], dtype=np.float32)
    b0 = bias[0].reshape(O, OH * OW)[:, 0].copy()

    if not _structure_ok(weight, w0, bias, b0):
        # Unstructured weight: fall back to the dense matmul on host.
        x2 = x.reshape(B, -1)
        w2 = np.asarray(weight, dtype=np.float32).reshape(O * OH * OW, -1)
        return (x2 @ w2.T + bias).reshape(B, O, OH, OW).astype(np.float32)

    _install_trace_shim()

    # host layout prep --------------------------------------------------
    # xq[core][hp, c, wp, b, h2, w2] = x[core*8 + b, c, 2*h2+hp, 2*w2+wp]
    xs = x.reshape(NCORES, BPC, C, HB, 2, WB, 2)
    xq = np.ascontiguousarray(xs.transpose(0, 4, 2, 6, 1, 3, 5)).reshape(NCORES, XQ_LEN)
    xqp = np.zeros((NCORES, XQ_PAD), dtype=np.float32)
    xqp[:, :XQ_LEN] = xq
    # X[(p), b*1024 + t] = xq[src_off[p] + b*1024 + t]
    idx = (_SRC_OFF[:, None, None]
           + np.arange(BPC)[None, :, None] * IMG
           + np.arange(IMG)[None, None, :])
    import ml_dtypes
    bf = ml_dtypes.bfloat16
    xbig = np.zeros((NCORES, NPARTB, BPC * IMG), dtype=bf)
    xbig[:, :NPART] = xqp[:, idx.reshape(NPART, -1)]
    xbig[:, NPART] = np.float32(1.0)  # folded-bias ones row; 76..79 stay 0

    # w0r[p, o] with p ordered (khp, kwp, khh, kwh, c); row 75 = b0 (bias
    # rides the ones row); cols 16..31 zero
    w0f = np.zeros((NPARTB, 32), dtype=np.float32)
    for p, (c, kh, kw) in enumerate(_PART_CKHKW):
        w0f[p, :O] = w0[:, c, kh, kw]
    w0f[NPART, :O] = b0
    w0r = w0f.astype(bf)

    # device run --------------------------------------------------------
    if "nc" not in _NC_CACHE:
        _NC_CACHE["nc"] = _build_nc()
    nc = _NC_CACHE["nc"]

    from concourse.bass_utils import run_bass_kernel_spmd

    in_maps = []
    for i in range(NCORES):
        m = {}
        for c, (s, e) in enumerate(CHUNKS):
            sl = xbig[i, :, IMG * s:IMG * e]
            if c == 0:
                sl = np.concatenate([sl, w0r], axis=1)
            m[f"xt{c}"] = np.ascontiguousarray(sl)
        in_maps.append(m)
    res = run_bass_kernel_spmd(nc, in_maps, core_ids=list(range(NCORES)))
    LAST_RESULT = res

    out = np.empty((B, O, OH, OW), dtype=np.float32)
    for i in range(NCORES):
        # device out: [g, 32*j + o, oh'*30 + ow] with
        # b = 2*g + (j>>1), oh = 15*(j&1) + oh'
        dv = res.results[i]["out"].astype(np.float32).reshape(4, 4, 32, 15, OW)
        for g in range(4):
            for j in range(4):
                b, hs = 2 * g + (j >> 1), j & 1
                out[i * BPC + b, :, 15 * hs:15 * hs + 15, :] = dv[g, j, :O]
    return out


# revision 54
# speedup vs baseline: 1.1000x; 1.0167x over previous
"""Conv2d-via-Linear Trainium2 kernel.

The problem's [16,30,30,3,64,64] weight is (for the reference's
setup_inputs) a structured-sparse replication of a single 5x5/stride-2
conv kernel w0 [16,3,5,5]:  big[:, oh, ow, :, 2oh:2oh+5, 2ow:2ow+5] = w0.
So out = x2 @ w2.T + bias is exactly Conv2d(x, w0, stride=2) + b0.

Device strategy (8 NeuronCores, batch-parallel, 8 images per core):
  - Host lays out each core's batch shard as a 75-row im2col operand
    X[(c,kh,kw), (b, oh, ow)] with the (kh>>1, kw>>1) spatial shifts baked
    in as flat offsets; a 76th all-ones row folds the bias into the
    matmul (weight row 75 holds b0).
  - Single-pass bf16 (PSUM accumulates fp32; rel err ~3e-3, well under
    the 2e-2 gate) instead of a 3-pass hi/lo split: halves input bytes
    and cuts matmul count 3x.
  - The DGE splits a DMA's descriptors (one per partition row) into
    equal contiguous blocks across k engines where k = the largest
    divisor of the row count that is <= 16. 76 rows -> 4 engines (76 =
    4*19, ~90 GB/s); 80 rows -> 16 engines (360 GB/s). So all loads pad
    the partition dim to 80 (rows 76..79 zero weights/data).
  - Input split into 4 chunks (2 images each, [80, 4KB rows])
    alternating the sync/scalar HWDGE rings so matmuls pipeline behind
    the stream (~230 GB/s effective; the stream rate is insensitive to
    descriptor size, so fine chunks win); the [80,32] weight block
    rides as trailing columns of chunk 0.
  - One fp32 matmul (K=76, M=32, cols 16..31 zero) per (image, oh-half)
    chunk; PE column-tiling packs 4 chunks into the 4 col-groups of one
    PSUM bank.
  - Evacuation casts PSUM fp32 -> SBUF bf16 (bias already folded in),
    alternating DVE/ACT engines; each [128,450] group DMAs out as soon
    as its evac lands; the host keeps rows :16 of each 32-group.

If the weight/bias do not have the replicated-conv structure (never the
case for the real reference inputs), falls back to the dense matmul on
host so the result is still correct.
"""

import numpy as np

B, C, H, W = 64, 3, 64, 64
O, KK, S = 16, 5, 2
OH = OW = 30
NCORES = 8
BPC = B // NCORES  # images per core

HB = H // 2  # 32  (h2 dim)
WB = W // 2  # 32  (w2 dim)
XQ_LEN = 2 * C * 2 * BPC * HB * WB  # 98304
XQ_PAD = XQ_LEN + 512  # shifted reads spill <= 2*256+1 past plane ends

# (khp, kwp) -> partition-block layout. kh = 2*khh + khp, kw = 2*kwh + kwp.
_BLOCKS = []
_p0 = 0
for _khp in (0, 1):
    for _kwp in (0, 1):
        _nkh = 3 - _khp
        _nkw = 3 - _kwp
        _BLOCKS.append((_khp, _kwp, _p0, _nkh, _nkw))
        _p0 += _nkh * _nkw * C
NPART = _p0  # 75
NPARTB = 80  # 75 taps + ones row (folded bias) + 4 zero pad rows; 80 = 16*5
             # so the DGE spreads every load across all 16 DMA engines

# element strides inside flat xq [hp, c, wp, b, h2, w2]
_ST_WP = BPC * HB * WB        # 8192
_ST_C = 2 * _ST_WP            # 16384
_ST_HP = C * _ST_C            # 49152

# per-partition source offsets into flat xq, and (c, kh, kw) per partition
_SRC_OFF = np.empty(NPART, dtype=np.int64)
_PART_CKHKW = []
for _khp, _kwp, _pp0, _nkh, _nkw in _BLOCKS:
    _pi = _pp0
    for _khh in range(_nkh):
        for _kwh in range(_nkw):
            for _c in range(C):
                _SRC_OFF[_pi] = (_khp * _ST_HP + _c * _ST_C + _kwp * _ST_WP
                                 + _khh * HB + _kwh)
                _PART_CKHKW.append((_c, 2 * _khh + _khp, 2 * _kwh + _kwp))
                _pi += 1

IMG = HB * WB  # 1024 elems per image per partition
# last two chunks are single images: their [64,450] outputs halve the
# critical-tail trigger generation and final transfer
CHUNKS = [(0, 2), (2, 4), (4, 6), (6, 7), (7, 8)]  # image ranges
NCHUNK = len(CHUNKS)

_NC_CACHE = {}
LAST_RESULT = None


def _install_trace_shim():
    """Make bass_utils' trace path importable even when antenv.axon_hooks
    is absent (it is in this container). Harmless if tracing is off."""
    import sys, types
    try:
        import antenv.axon_hooks  # noqa: F401
        return
    except ImportError:
        pass
    mod = types.ModuleType("antenv.axon_hooks")
    hook = [None]
    mod.set_axon_ntff_profile_hook = lambda h: hook.__setitem__(0, h)
    mod.get_axon_ntff_profile_hook = lambda: hook[0]
    sys.modules["antenv.axon_hooks"] = mod
    try:
        from trn_agent_boot.trn_boot import _ntff_profile_via_ctypes
        hook[0] = _ntff_profile_via_ctypes("/opt/axon/libaxon_pjrt.so")
    except Exception:
        pass


def _structure_ok(weight, w0, bias, b0):
    """Exact check that `weight` is w0 replicated per output position and
    everything else zero, and that bias is b0 repeated per position."""
    try:
        from numpy.lib.stride_tricks import as_strided
        s = weight.strides
        blocks = as_strided(
            weight,
            shape=(OH, OW, O, C, KK, KK),
            strides=(s[1] + S * s[4], s[2] + S * s[5], s[0], s[3], s[4], s[5]),
        )
        if not (blocks == w0[None, None]).all():
            return False
        if np.count_nonzero(weight) != OH * OW * np.count_nonzero(w0):
            return False
        if not (bias[0].reshape(O, OH * OW) == b0[:, None]).all():
            return False
        return True
    except Exception:
        return False


def _build_nc():
    import concourse.mybir as mybir
    import concourse.tile as tile
    from concourse import bacc

    f32 = mybir.dt.float32
    bf16 = mybir.dt.bfloat16
    nc = bacc.Bacc(None, target_bir_lowering=False)
    with tile.TileContext(nc) as tc:
        with tc.tile_pool(name="dram", bufs=1, space="DRAM") as dram:
            # chunk 0 carries the [80,32] weight block as trailing columns
            xt_d = [
                dram.tile([NPARTB, IMG * (e - s) + (32 if c == 0 else 0)],
                          bf16, kind="ExternalInput", name=f"xt{c}",
                          uniquify=False)
                for c, (s, e) in enumerate(CHUNKS)
            ]
            out = dram.tile([3, 128, 15 * OW], bf16, kind="ExternalOutput",
                            name="out", uniquify=False)
            outb = dram.tile([2, 64, 15 * OW], bf16, kind="ExternalOutput",
                             name="outb", uniquify=False)

            with (
                tc.tile_pool(name="xdata", bufs=1) as xpool,
                tc.tile_pool(name="evac", bufs=4) as evacp,
                tc.tile_pool(name="psum", bufs=4, space="PSUM") as psump,
            ):
                # everything on the sync ring: strict global FIFO means the
                # output transfers queue behind ALL input chunks, so they
                # can never interleave with (and drag out) the input stream
                xts = []
                for c in range(NCHUNK):
                    t = xpool.tile(list(xt_d[c].shape), bf16, name=f"xs{c}")
                    nc.sync.dma_start(t[:], xt_d[c][:])
                    xts.append(t)
                wsb = xts[0][:, 2 * IMG:2 * IMG + 32]  # [80,32], 16..31 zero

                for g in range(NCHUNK):
                    nb = CHUNKS[g][1] - CHUNKS[g][0]
                    xv = xts[g][:, 0:nb * IMG].rearrange(
                        "p (b h w) -> p b h w", b=nb, h=HB, w=WB)
                    ps = psump.tile([128, 15, OW], f32, tag="ps")
                    for j in range(2 * nb):
                        bl, hs = j >> 1, j & 1
                        rhs = xv[:, bl, 15 * hs:15 * hs + 15, 0:OW]
                        nc.tensor.matmul(
                            ps[32 * j:32 * j + 32],
                            wsb,
                            rhs,
                            start=True,
                            stop=True,
                            skip_group_check=True,
                            tile_position=(0, 32 * j),
                        )
                    # evacuate with fp32->bf16 cast (bias already folded in);
                    # all on DVE: ACT copy would pull in a 1.3us table load
                    src = ps[:].rearrange("p a b -> p (a b)")
                    if nb == 2:
                        ev = evacp.tile([128, 15 * OW], bf16, tag="ev")
                        nc.vector.tensor_scalar_add(ev[:], src, 0.0)
                        nc.sync.dma_start(out[g, :, :], ev[:])
                    else:
                        ev = evacp.tile([64, 15 * OW], bf16, tag="ev64")
                        nc.vector.tensor_scalar_add(ev[:], src[0:64, :], 0.0)
                        nc.sync.dma_start(outb[g - 3, :, :], ev[:])
    nc.compile()
    return nc


def kernel(x, weight, bias):
    global LAST_RESULT
    x = np.ascontiguousarray(np.asarray(x), dtype=np.float32)
    weight = np.asarray(weight)
    bias = np.ascontiguousarray(np.asarray(bias), dtype=np.float32)

    w0 = np.ascontiguousarray(weight[:, 0, 0, :, :KK, :KK], dtype=np.float32)
    b0 = bias[0].reshape(O, OH * OW)[:, 0].copy()

    if not _structure_ok(weight, w0, bias, b0):
        # Unstructured weight: fall back to the dense matmul on host.
        x2 = x.reshape(B, -1)
        w2 = np.asarray(weight, dtype=np.float32).reshape(O * OH * OW, -1)
        return (x2 @ w2.T + bias).reshape(B, O, OH, OW).astype(np.float32)

    _install_trace_shim()

    # host layout prep --------------------------------------------------
    # xq[core][hp, c, wp, b, h2, w2] = x[core*8 + b, c, 2*h2+hp, 2*w2+wp]
    xs = x.reshape(NCORES, BPC, C, HB, 2, WB, 2)
    xq = np.ascontiguousarray(xs.transpose(0, 4, 2, 6, 1, 3, 5)).reshape(NCORES, XQ_LEN)
    xqp = np.zeros((NCORES, XQ_PAD), dtype=np.float32)
    xqp[:, :XQ_LEN] = xq
    # X[(p), b*1024 + t] = xq[src_off[p] + b*1024 + t]
    idx = (_SRC_OFF[:, None, None]
           + np.arange(BPC)[None, :, None] * IMG
           + np.arange(IMG)[None, None, :])
    import ml_dtypes
    bf = ml_dtypes.bfloat16
    xbig = np.zeros((NCORES, NPARTB, BPC * IMG), dtype=bf)
    xbig[:, :NPART] = xqp[:, idx.reshape(NPART, -1)]
    xbig[:, NPART] = np.float32(1.0)  # folded-bias ones row; 76..79 stay 0

    # w0r[p, o] with p ordered (khp, kwp, khh, kwh, c); row 75 = b0 (bias
    # rides the ones row); cols 16..31 zero
    w0f = np.zeros((NPARTB, 32), dtype=np.float32)
    for p, (c, kh, kw) in enumerate(_PART_CKHKW):
        w0f[p, :O] = w0[:, c, kh, kw]
    w0f[NPART, :O] = b0
    w0r = w0f.astype(bf)

    # device run --------------------------------------------------------
    if "nc" not in _NC_CACHE:
        _NC_CACHE["nc"] = _build_nc()
    nc = _NC_CACHE["nc"]

    from concourse.bass_utils import run_bass_kernel_spmd

    in_maps = []
    for i in range(NCORES):
        m = {}
        for c, (s, e) in enumerate(CHUNKS):
            sl = xbig[i, :, IMG * s:IMG * e]
            if c == 0:
                sl = np.concatenate([sl, w0r], axis=1)
            m[f"xt{c}"] = np.ascontiguousarray(sl)
        in_maps.append(m)
    res = run_bass_kernel_spmd(nc, in_maps, core_ids=list(range(NCORES)))
    LAST_RESULT = res

    out = np.empty((B, O, OH, OW), dtype=np.float32)
    for i in range(NCORES):
        # out: [g, 32*j + o, oh'*30 + ow], b = 2*g + (j>>1), oh = 15*(j&1)
        dv = res.results[i]["out"].astype(np.float32).reshape(3, 4, 32, 15, OW)
        for g in range(3):
            for j in range(4):
                b, hs = 2 * g + (j >> 1), j & 1
                out[i * BPC + b, :, 15 * hs:15 * hs + 15, :] = dv[g, j, :O]
        # outb: [k, 32*hs + o, oh'*30 + ow], b = 6 + k
        db = res.results[i]["outb"].astype(np.float32).reshape(2, 2, 32, 15, OW)
        for k in range(2):
            for hs in range(2):
                out[i * BPC + 6 + k, :, 15 * hs:15 * hs + 15, :] = db[k, hs, :O]
    return out
